# revision 17
# baseline (speedup 1.0000x reference)
"""GQA attention with BitLinear projections, RMSNorm+RoPE, tanh softcap.

Sharding: 8 cores = batch(2) x kv-group(4). Each core handles one batch
element and one kv head (+ its 4 query heads), computes a partial o-proj
against its 256 columns of wo, and the host sums the 8 partials.
"""

import sys

if "/opt/trn_rl_repo" not in sys.path:
    sys.path.insert(0, "/opt/trn_rl_repo")

import numpy as np

import concourse.bass as bass
import concourse.mybir as mybir
import concourse.tile as tile
from concourse import bacc
from concourse.bass_utils import run_bass_kernel_spmd
from concourse.masks import make_identity

B, T, D, H, KVH, HD = 2, 2048, 1024, 16, 4, 64
HEADS_PER_CORE = H // KVH  # 4
DC = HEADS_PER_CORE * HD  # 256 q-proj dim per core
N_CORES = 8
SOFTCAP = 50.0
EPS = 1e-6
P = 128
HH = HD // 2

F32 = mybir.dt.float32
F32R = mybir.dt.float32r
I32 = mybir.dt.int32

QK_DT = F32R   # qT/kT tiles
PV_DT = F32R   # p and v tiles
PJ_DT = F32R   # x / projection weights
MAGIC = 0x5F375A86

_CACHE = {}


def _build(t_len, mask_mode):
    """mask_mode: 'none' | 'causal' | 'general'."""
    nt = t_len // P          # 128-row t slices
    ntc = t_len // 512       # 512-col t tiles
    ntp = max(t_len // 1024, 1)  # t chunk pairs (1024)
    tc_per_tp = ntc // ntp
    nt_per_tp = nt // ntp
    ns = t_len // P          # s chunks
    KO = D // P              # 8 contraction chunks
    AOP = mybir.AluOpType

    nc = bacc.Bacc(None, target_bir_lowering=False)

    xT_d = nc.dram_tensor("xT", [D, t_len], PJ_DT, kind="ExternalInput")
    wqT_d = nc.dram_tensor("wqT", [D, DC], PJ_DT, kind="ExternalInput")
    wkvT_d = nc.dram_tensor("wkvT", [D, 2 * HD], PJ_DT, kind="ExternalInput")
    woT_d = nc.dram_tensor("woT", [DC, D], PJ_DT, kind="ExternalInput")
    cosq_d = nc.dram_tensor("cosq", [t_len, HD], F32, kind="ExternalInput")
    sinq_d = nc.dram_tensor("sinq", [t_len, HD], F32, kind="ExternalInput")
    cosk_d = nc.dram_tensor("cosk", [t_len, HD], F32, kind="ExternalInput")
    sink_d = nc.dram_tensor("sink", [t_len, HD], F32, kind="ExternalInput")
    if mask_mode != "none":
        # mask transposed to [s, t] and divided by SOFTCAP on host
        maskT_d = nc.dram_tensor("maskT", [t_len, t_len], F32,
                                 kind="ExternalInput")
    y_d = nc.dram_tensor("y", [t_len, D], F32, kind="ExternalOutput")

    AF = mybir.ActivationFunctionType

    with tile.TileContext(nc) as tc:
        with (
            tc.tile_pool(name="const", bufs=1) as constp,
            tc.tile_pool(name="big", bufs=1) as bigp,
            tc.tile_pool(name="work", bufs=2) as workp,
            tc.tile_pool(name="normp", bufs=2) as normp,
            tc.tile_pool(name="tbp", bufs=2) as tbp,
            tc.tile_pool(name="pbp", bufs=2) as pbp,
            tc.tile_pool(name="outp", bufs=1) as outp,
            tc.tile_pool(name="stage", bufs=2) as stagep,
            tc.tile_pool(name="psum_s", bufs=4, space="PSUM") as psum_s,
            tc.tile_pool(name="psum_qk", bufs=1, space="PSUM") as psum_qk,
        ):
            ident = constp.tile([P, P], F32)
            make_identity(nc, ident)

            # ---- persistent loads ----
            xT_sb = bigp.tile([P, KO, t_len], PJ_DT, tag="xT")
            nc.sync.dma_start(xT_sb[:], xT_d.rearrange("(o p) t -> p o t", p=P))
            wqT_sb = bigp.tile([P, KO, DC], PJ_DT, tag="wqT")
            nc.sync.dma_start(wqT_sb[:], wqT_d.rearrange("(o p) d -> p o d", p=P))
            wkvT_sb = bigp.tile([P, KO, 2 * HD], PJ_DT, tag="wkvT")
            nc.sync.dma_start(wkvT_sb[:], wkvT_d.rearrange("(o p) d -> p o d", p=P))
            woT_sb = bigp.tile([P, 2, D], PJ_DT, tag="woT")
            nc.sync.dma_start(woT_sb[:], woT_d.rearrange("(o p) e -> p o e", p=P))
            cs_sb = {}
            for name, dram in (("cq", cosq_d), ("sq", sinq_d),
                               ("ck", cosk_d), ("sk", sink_d)):
                cs_sb[name] = bigp.tile([P, nt, HD], F32, tag=name, name=name)
                nc.sync.dma_start(cs_sb[name][:],
                                  dram.rearrange("(o p) d -> p o d", p=P))

            qT_tp = [bigp.tile([P, 2, nt_per_tp * P], QK_DT, tag=f"qT{tp}",
                               name=f"qT{tp}") for tp in range(ntp)]
            kT_sb = bigp.tile([P, t_len], QK_DT, tag="kT")
            v_sb = bigp.tile([P, ns, HD + 2], PV_DT, tag="v")
            nc.vector.memset(v_sb[:].bitcast(F32), 1.0)

            magic = constp.tile([P, 8], I32, name="magic")
            nc.vector.memset(magic[:], MAGIC)

            def rsqrt_newton(m_ap, y_tile, width):
                """y = rsqrt(m) via bit-trick seed + 3 Newton iters (DVE)."""
                y_int = y_tile[:].bitcast(I32)
                nc.vector.tensor_scalar(y_int, m_ap.bitcast(I32), 1, None,
                                        op0=AOP.logical_shift_right)
                nc.vector.tensor_tensor(y_int, magic[:, 0:width], y_int,
                                        op=AOP.subtract)
                t1 = normp.tile([P, 8], F32, tag="t1")
                t1 = t1[:, 0:width]
                for _ in range(3):
                    nc.vector.tensor_tensor(t1, y_tile[:], y_tile[:],
                                            op=AOP.mult)
                    nc.vector.tensor_tensor(t1, m_ap, t1, op=AOP.mult)
                    nc.vector.tensor_scalar(t1, t1, -0.5, 1.5,
                                            op0=AOP.mult, op1=AOP.add)
                    nc.vector.tensor_tensor(y_tile[:], y_tile[:], t1,
                                            op=AOP.mult)

            def rope(dst, src, cn, sn, i, nh):
                """dst = src*cos + rotate_half(src)*sin; src [P, nh, HD]."""
                ta = workp.tile([P, HEADS_PER_CORE, HH], F32, tag="ta")
                ta = ta[:, 0:nh, :]
                bc = lambda ap: ap.to_broadcast((P, nh, HH)) if nh > 1 else ap
                c_lo = bc(cs_sb[cn][:, i:i + 1, 0:HH])
                s_lo = bc(cs_sb[sn][:, i:i + 1, 0:HH])
                c_hi = bc(cs_sb[cn][:, i:i + 1, HH:HD])
                s_hi = bc(cs_sb[sn][:, i:i + 1, HH:HD])
                nc.vector.tensor_tensor(dst[:, :, 0:HH], src[:, :, 0:HH], c_lo,
                                        op=AOP.mult)
                nc.vector.tensor_tensor(ta, src[:, :, HH:HD], s_lo, op=AOP.mult)
                nc.vector.tensor_tensor(dst[:, :, 0:HH], dst[:, :, 0:HH], ta,
                                        op=AOP.subtract)
                nc.vector.tensor_tensor(dst[:, :, HH:HD], src[:, :, HH:HD],
                                        c_hi, op=AOP.mult)
                nc.vector.tensor_tensor(ta, src[:, :, 0:HH], s_hi, op=AOP.mult)
                nc.vector.tensor_tensor(dst[:, :, HH:HD], dst[:, :, HH:HD], ta,
                                        op=AOP.add)

            def kv_slice(i):
                """KV projection + k rmsnorm/rope + kT dup + v for slice i.

                Runs in the serial prefix: psum->sbuf copies and squares go
                on the otherwise-idle ScalarE.
                """
                kv_ps = psum_s.tile([P, 2 * HD], F32, tag="ps")
                for ko in range(KO):
                    nc.tensor.matmul(kv_ps[:], xT_sb[:, ko, i * P:(i + 1) * P],
                                     wkvT_sb[:, ko, :],
                                     start=(ko == 0), stop=(ko == KO - 1))
                kv_sb = workp.tile([P, 2 * HD], F32, tag="kvsb")
                nc.scalar.copy(kv_sb[:], kv_ps[:])
                nc.vector.tensor_copy(v_sb[:, i, 0:HD], kv_ps[:, HD:2 * HD])

                scrk = normp.tile([P, HD], F32, tag="scrk")
                nc.scalar.square(scrk[:], kv_sb[:, 0:HD])
                m_k = normp.tile([P, 1], F32, tag="mk")
                nc.vector.tensor_reduce(m_k[:], scrk[:],
                                        axis=mybir.AxisListType.X, op=AOP.add)
                nc.vector.tensor_scalar(m_k[:], m_k[:], 1.0 / HD, EPS,
                                        op0=AOP.mult, op1=AOP.add)
                yk = normp.tile([P, 1], F32, tag="yk")
                rsqrt_newton(m_k[:], yk, 1)
                kn = workp.tile([P, 1, HD], F32, tag="kn")
                nc.vector.tensor_scalar(kn[:, 0, :], kv_sb[:, 0:HD], yk[:],
                                        None, op0=AOP.mult)
                rk = workp.tile([P, 1, HD], F32, tag="rk")
                rope(rk, kn, "ck", "sk", i, 1)
                tk_ps = psum_s.tile([HD, P], F32, tag="ps")
                nc.tensor.transpose(tk_ps[:], rk[:, 0, :], ident[:])
                nc.vector.tensor_copy(kT_sb[0:HD, i * P:(i + 1) * P], tk_ps[:])
                nc.vector.tensor_copy(kT_sb[HD:P, i * P:(i + 1) * P], tk_ps[:])

            def q_slice(i, qT_dst, di):
                """Q projection + rmsnorm/rope + transpose for slice i.
                Writes qT_dst[:, :, di*128:...]. DVE-heavy (overlaps attn)."""
                q_ps = psum_s.tile([P, DC], F32, tag="ps")
                for ko in range(KO):
                    nc.tensor.matmul(q_ps[:], xT_sb[:, ko, i * P:(i + 1) * P],
                                     wqT_sb[:, ko, :],
                                     start=(ko == 0), stop=(ko == KO - 1))
                q_sb = workp.tile([P, DC], F32, tag="qsb")
                nc.vector.tensor_copy(q_sb[:], q_ps[:])
                scr = normp.tile([P, HEADS_PER_CORE, HD], F32, tag="scr")
                nc.vector.tensor_tensor(
                    scr[:], q_sb[:].rearrange("p (h d) -> p h d", d=HD),
                    q_sb[:].rearrange("p (h d) -> p h d", d=HD), op=AOP.mult)
                m_q = normp.tile([P, HEADS_PER_CORE], F32, tag="mq")
                nc.vector.tensor_reduce(m_q[:], scr[:],
                                        axis=mybir.AxisListType.X, op=AOP.add)
                nc.vector.tensor_scalar(m_q[:], m_q[:], 1.0 / HD, EPS,
                                        op0=AOP.mult, op1=AOP.add)
                yq = normp.tile([P, HEADS_PER_CORE], F32, tag="yq")
                rsqrt_newton(m_q[:], yq, HEADS_PER_CORE)
                qn = workp.tile([P, HEADS_PER_CORE, HD], F32, tag="qn")
                for h in range(HEADS_PER_CORE):
                    nc.vector.tensor_scalar(qn[:, h, :],
                                            q_sb[:, h * HD:(h + 1) * HD],
                                            yq[:, h:h + 1], None, op0=AOP.mult)
                rq = workp.tile([P, HEADS_PER_CORE, HD], F32, tag="rq")
                rope(rq, qn, "cq", "sq", i, HEADS_PER_CORE)
                for mc in range(2):
                    t_ps = psum_s.tile([P, P], F32, tag="ps")
                    nc.tensor.transpose(t_ps[:], rq[:, 2 * mc:2 * mc + 2, :],
                                        ident[:])
                    nc.vector.tensor_copy(qT_dst[:, mc, di * P:(di + 1) * P],
                                          t_ps[:])

            def attn(hp, tp):
                """Attention for head pair hp over t chunk tp.

                Returns outT tile [128, tw]: rows 0-63 head 2hp, 64-127
                head 2hp+1 (o-proj lhsT layout)."""
                t0 = tp * tc_per_tp * 512
                tw = tc_per_tp * 512
                qT_sb = qT_tp[tp]
                ow = outp.tile([P, tw], PV_DT, tag=f"ot_{hp}_{tp}",
                               name=f"ot_{hp}_{tp}")
                pv_ps = [psum_s.tile([P, 512], F32, tag="ps",
                                     name=f"pvps{_j}")
                         for _j in range(2 * tc_per_tp)]
                if mask_mode == "causal":
                    s_list = [s for s in range(ns) if s * P <= t0 + tw - 1]
                else:
                    s_list = list(range(ns))
                for si, s in enumerate(s_list):
                    qk_ps = psum_qk.tile([P, 2, tc_per_tp, 512], F32, tag="qk")
                    for j in range(2):
                        for tci in range(tc_per_tp):
                            nc.tensor.matmul(
                                qk_ps[:, j, tci, :],
                                kT_sb[HD * j:HD * (j + 1), s * P:(s + 1) * P],
                                qT_sb[HD * j:HD * (j + 1), hp,
                                      tci * 512:(tci + 1) * 512],
                                start=True, stop=True,
                                tile_position=(HD * j, 0))
                    tb = tbp.tile([P, 2, tc_per_tp, 512], F32, tag="tb")
                    nc.scalar.activation(tb[:], qk_ps[:], AF.Tanh,
                                         scale=1.0 / (8.0 * SOFTCAP))
                    if mask_mode != "none":
                        if mask_mode == "general" or s * P + P > t0:
                            mt = stagep.tile([P, tc_per_tp, 512], F32, tag="mt")
                            nc.sync.dma_start(
                                mt[:], maskT_d[s * P:(s + 1) * P, t0:t0 + tw]
                                .rearrange("p (c f) -> p c f", f=512))
                            nc.vector.tensor_tensor(
                                tb[:], tb[:],
                                mt[:, None, :, :].to_broadcast(
                                    (P, 2, tc_per_tp, 512)),
                                op=AOP.add)
                    pb = pbp.tile([P, 2, tc_per_tp, 512], PV_DT, tag="pb")
                    nc.scalar.activation(pb[:], tb[:], AF.Exp, scale=SOFTCAP)
                    for j in range(2):
                        for tci in range(tc_per_tp):
                            nc.tensor.matmul(
                                pv_ps[j * tc_per_tp + tci][0:HD + 1, :],
                                v_sb[:, s, 0:HD + 1],
                                pb[:, j, tci, :],
                                start=(si == 0), stop=(si == len(s_list) - 1))
                # normalize -> outT
                for j in range(2):
                    for tci in range(tc_per_tp):
                        jt = j * tc_per_tp + tci
                        rb = stagep.tile([HD, 512], F32, tag="rb", bufs=2)
                        nc.vector.reciprocal(rb[0:1, :],
                                             pv_ps[jt][HD:HD + 1, :])
                        nc.gpsimd.partition_broadcast(rb[:], rb[0:1, :],
                                                      channels=HD)
                        nc.vector.tensor_tensor(
                            ow[HD * j:HD * (j + 1),
                               tci * 512:(tci + 1) * 512],
                            pv_ps[jt][0:HD, :], rb[:], op=AOP.mult)
                return ow

            def oproj(ow_by_hp, tp):
                for ii in range(tc_per_tp * 4):
                    gi = tp * tc_per_tp * 4 + ii
                    for nh in range(2):
                        op_ps = psum_s.tile([P, 512], F32, tag="ps")
                        for ko in range(2):
                            nc.tensor.matmul(
                                op_ps[:],
                                ow_by_hp[ko][:, ii * P:(ii + 1) * P],
                                woT_sb[:, ko, nh * 512:(nh + 1) * 512],
                                start=(ko == 0), stop=(ko == 1))
                        o_sb = stagep.tile([P, 512], F32, tag="osb", bufs=3)
                        nc.vector.tensor_copy(o_sb[:], op_ps[:])
                        nc.sync.dma_start(
                            y_d.rearrange("(o p) e -> p o e",
                                          p=P)[:, gi, nh * 512:(nh + 1) * 512],
                            o_sb[:])

            # ---- emission: kv first, then per-tp q-proj / attn / o-proj ----
            for i in range(nt):
                kv_slice(i)
            for di in range(nt_per_tp):
                q_slice(di, qT_tp[0], di)
            prev = None
            for tp in range(ntp):
                ow_by_hp = []
                for hp in range(2):
                    ow_by_hp.append(attn(hp, tp))
                    if hp == 0:
                        if tp + 1 < ntp:
                            for di in range(nt_per_tp):
                                q_slice((tp + 1) * nt_per_tp + di,
                                        qT_tp[tp + 1], di)
                        if prev is not None:
                            oproj(prev, tp - 1)
                prev = ow_by_hp
            oproj(prev, ntp - 1)

    nc.finalize()
    return nc


def _get_nc(t_len, mask_mode):
    key = (t_len, mask_mode)
    if key not in _CACHE:
        _CACHE[key] = _build(t_len, mask_mode)
    return _CACHE[key]


def _host_prep(x, cos, sin, mask, wq, wk, wv, wo, q_norm_w, k_norm_w, t_len):
    f = np.float32
    wq, wk, wv, wo = (np.asarray(a, f) for a in (wq, wk, wv, wo))
    x = np.asarray(x, f)
    cos, sin = np.asarray(cos, f), np.asarray(sin, f)
    qw, kw = np.asarray(q_norm_w, f), np.asarray(k_norm_w, f)

    def eff(w):
        alpha = np.mean(np.abs(w), dtype=f)
        return (np.sign(w) * alpha).astype(f)

    wqe, wke, wve, woe = eff(wq), eff(wk), eff(wv), eff(wo)

    qw_sw = np.concatenate([qw[HH:], qw[:HH]])
    kw_sw = np.concatenate([kw[HH:], kw[:HH]])
    cosq = np.ascontiguousarray(cos * qw[None, :])
    sinq = np.ascontiguousarray(sin * qw_sw[None, :])
    cosk = np.ascontiguousarray(cos * kw[None, :])
    sink = np.ascontiguousarray(sin * kw_sw[None, :])

    m2 = np.asarray(mask, f).reshape(t_len, t_len)
    if not np.any(m2):
        mask_mode = "none"
        maskT = None
    else:
        causal = np.array_equal(
            m2, np.where(np.tril(np.ones((t_len, t_len), bool)), f(0), f(-1e9)))
        mask_mode = "causal" if causal else "general"
        maskT = np.ascontiguousarray(m2.T) / f(SOFTCAP)

    in_maps = []
    for c in range(N_CORES):
        b, g = divmod(c, KVH)
        im = {
            "xT": np.ascontiguousarray(x[b].T),
            "wqT": np.ascontiguousarray(wqe[g * DC:(g + 1) * DC, :].T),
            "wkvT": np.ascontiguousarray(
                np.concatenate([wke[g * HD:(g + 1) * HD, :],
                                wve[g * HD:(g + 1) * HD, :]], axis=0).T),
            "woT": np.ascontiguousarray(woe.T[g * DC:(g + 1) * DC, :]),
            "cosq": cosq, "sinq": sinq, "cosk": cosk, "sink": sink,
        }
        if maskT is not None:
            im["maskT"] = maskT
        in_maps.append(im)
    return in_maps, mask_mode


def kernel(x, cos, sin, mask, wq, wk, wv, wo, q_norm_w, k_norm_w,
           _trace=False, _t_len=T):
    in_maps, mask_mode = _host_prep(x, cos, sin, mask, wq, wk, wv, wo,
                                    q_norm_w, k_norm_w, _t_len)
    nc = _get_nc(_t_len, mask_mode)
    res = run_bass_kernel_spmd(nc, in_maps, core_ids=list(range(N_CORES)),
                               trace=_trace)
    out = np.zeros((B, _t_len, D), np.float32)
    for c in range(N_CORES):
        b = c // KVH
        out[b] += res.results[c]["y"]
    if _trace:
        kernel._last = res
    return out


# revision 20
# speedup vs baseline: 1.2507x; 1.2507x over previous
"""GQA attention with BitLinear projections, RMSNorm+RoPE, tanh softcap.

Sharding: 8 cores = batch(2) x kv-group(4). Each core handles one batch
element and one kv head (+ its 4 query heads), computes a partial o-proj
against its 256 columns of wo, and the host sums the 8 partials.
"""

import sys

if "/opt/trn_rl_repo" not in sys.path:
    sys.path.insert(0, "/opt/trn_rl_repo")

import numpy as np

import concourse.bass as bass
import concourse.mybir as mybir
import concourse.tile as tile
from concourse import bacc
from concourse.bass_utils import run_bass_kernel_spmd
from concourse.masks import make_identity

B, T, D, H, KVH, HD = 2, 2048, 1024, 16, 4, 64
HEADS_PER_CORE = H // KVH  # 4
DC = HEADS_PER_CORE * HD  # 256 q-proj dim per core
N_CORES = 8
SOFTCAP = 50.0
EPS = 1e-6
P = 128
HH = HD // 2

F32 = mybir.dt.float32
F32R = mybir.dt.float32r
I32 = mybir.dt.int32

QK_DT = F32R   # qT/kT tiles
PV_DT = F32R   # p and v tiles
PJ_DT = F32R   # x / projection weights
MAGIC = 0x5F375A86

_CACHE = {}


def _build(t_len, mask_mode):
    """mask_mode: 'none' | 'causal' | 'general'."""
    nt = t_len // P          # 128-row t slices
    ntc = t_len // 512       # 512-col t tiles
    ntp = max(t_len // 1024, 1)  # t chunk pairs (1024)
    tc_per_tp = ntc // ntp
    nt_per_tp = nt // ntp
    ns = t_len // P          # s chunks
    KO = D // P              # 8 contraction chunks
    AOP = mybir.AluOpType

    nc = bacc.Bacc(None, target_bir_lowering=False)

    xT_d = nc.dram_tensor("xT", [D, t_len], PJ_DT, kind="ExternalInput")
    wqT_d = nc.dram_tensor("wqT", [D, DC], PJ_DT, kind="ExternalInput")
    wkvT_d = nc.dram_tensor("wkvT", [D, 2 * HD], PJ_DT, kind="ExternalInput")
    woT_d = nc.dram_tensor("woT", [DC, D], PJ_DT, kind="ExternalInput")
    cosq_d = nc.dram_tensor("cosq", [t_len, HD], F32, kind="ExternalInput")
    sinq_d = nc.dram_tensor("sinq", [t_len, HD], F32, kind="ExternalInput")
    cosk_d = nc.dram_tensor("cosk", [t_len, HD], F32, kind="ExternalInput")
    sink_d = nc.dram_tensor("sink", [t_len, HD], F32, kind="ExternalInput")
    if mask_mode != "none":
        # mask transposed to [s, t] and divided by SOFTCAP on host
        maskT_d = nc.dram_tensor("maskT", [t_len, t_len], F32,
                                 kind="ExternalInput")
    y_d = nc.dram_tensor("y", [t_len, D], F32, kind="ExternalOutput")

    AF = mybir.ActivationFunctionType

    with tile.TileContext(nc) as tc:
        with (
            tc.tile_pool(name="const", bufs=1) as constp,
            tc.tile_pool(name="big", bufs=1) as bigp,
            tc.tile_pool(name="work", bufs=2) as workp,
            tc.tile_pool(name="normp", bufs=2) as normp,
            tc.tile_pool(name="tbp", bufs=2) as tbp,
            tc.tile_pool(name="pbp", bufs=2) as pbp,
            tc.tile_pool(name="outp", bufs=1) as outp,
            tc.tile_pool(name="stage", bufs=2) as stagep,
            tc.tile_pool(name="psum_s", bufs=4, space="PSUM") as psum_s,
            tc.tile_pool(name="psum_qk", bufs=1, space="PSUM") as psum_qk,
        ):
            ident = constp.tile([P, P], F32)
            make_identity(nc, ident)

            # ---- persistent loads ----
            xT_sb = bigp.tile([P, KO, t_len], PJ_DT, tag="xT")
            nc.sync.dma_start(xT_sb[:], xT_d.rearrange("(o p) t -> p o t", p=P))
            wqT_sb = bigp.tile([P, KO, DC], PJ_DT, tag="wqT")
            nc.sync.dma_start(wqT_sb[:], wqT_d.rearrange("(o p) d -> p o d", p=P))
            wkvT_sb = bigp.tile([P, KO, 2 * HD], PJ_DT, tag="wkvT")
            nc.sync.dma_start(wkvT_sb[:], wkvT_d.rearrange("(o p) d -> p o d", p=P))
            woT_sb = bigp.tile([P, 2, D], PJ_DT, tag="woT")
            nc.sync.dma_start(woT_sb[:], woT_d.rearrange("(o p) e -> p o e", p=P))
            cs_sb = {}
            for name, dram in (("cq", cosq_d), ("sq", sinq_d),
                               ("ck", cosk_d), ("sk", sink_d)):
                cs_sb[name] = bigp.tile([P, nt, HD], F32, tag=name, name=name)
                nc.sync.dma_start(cs_sb[name][:],
                                  dram.rearrange("(o p) d -> p o d", p=P))

            qT_tp = [bigp.tile([P, 2, nt_per_tp * P], QK_DT, tag=f"qT{tp}",
                               name=f"qT{tp}") for tp in range(ntp)]
            kT_sb = bigp.tile([P, t_len], QK_DT, tag="kT")
            v_sb = bigp.tile([P, ns, HD + 2], PV_DT, tag="v")
            nc.vector.memset(v_sb[:].bitcast(F32), 1.0)

            magic = constp.tile([P, 16], I32, name="magic")
            nc.vector.memset(magic[:], MAGIC)

            def rsqrt_newton(m_ap, y_tile, width):
                """y = rsqrt(m) via bit-trick seed + 3 Newton iters (DVE)."""
                y_int = y_tile[:].bitcast(I32)
                nc.vector.tensor_scalar(y_int, m_ap.bitcast(I32), 1, None,
                                        op0=AOP.logical_shift_right)
                nc.vector.tensor_tensor(y_int, magic[:, 0:width], y_int,
                                        op=AOP.subtract)
                t1 = normp.tile([P, 16], F32, tag="t1")
                t1 = t1[:, 0:width]
                for _ in range(3):
                    nc.vector.tensor_tensor(t1, y_tile[:], y_tile[:],
                                            op=AOP.mult)
                    nc.vector.tensor_tensor(t1, m_ap, t1, op=AOP.mult)
                    nc.vector.tensor_scalar(t1, t1, -0.5, 1.5,
                                            op0=AOP.mult, op1=AOP.add)
                    nc.vector.tensor_tensor(y_tile[:], y_tile[:], t1,
                                            op=AOP.mult)

            def rope(dst, src, cn, sn, i, nh):
                """dst = src*cos + rotate_half(src)*sin; src [P, nh, HD]."""
                ta = workp.tile([P, HEADS_PER_CORE, HH], F32, tag="ta")
                ta = ta[:, 0:nh, :]
                bc = lambda ap: ap.to_broadcast((P, nh, HH)) if nh > 1 else ap
                c_lo = bc(cs_sb[cn][:, i:i + 1, 0:HH])
                s_lo = bc(cs_sb[sn][:, i:i + 1, 0:HH])
                c_hi = bc(cs_sb[cn][:, i:i + 1, HH:HD])
                s_hi = bc(cs_sb[sn][:, i:i + 1, HH:HD])
                nc.vector.tensor_tensor(dst[:, :, 0:HH], src[:, :, 0:HH], c_lo,
                                        op=AOP.mult)
                nc.vector.tensor_tensor(ta, src[:, :, HH:HD], s_lo, op=AOP.mult)
                nc.vector.tensor_tensor(dst[:, :, 0:HH], dst[:, :, 0:HH], ta,
                                        op=AOP.subtract)
                nc.vector.tensor_tensor(dst[:, :, HH:HD], src[:, :, HH:HD],
                                        c_hi, op=AOP.mult)
                nc.vector.tensor_tensor(ta, src[:, :, 0:HH], s_hi, op=AOP.mult)
                nc.vector.tensor_tensor(dst[:, :, HH:HD], dst[:, :, HH:HD], ta,
                                        op=AOP.add)

            def kv_batch(i0, nsl):
                """KV proj + k rmsnorm/rope + kT dup + v for slices
                [i0, i0+nsl); one batched Newton rsqrt for the whole batch.
                Copies/squares go on the otherwise-idle ScalarE."""
                kv_sbs = []
                m_k = normp.tile([P, 8], F32, tag="mk")
                for di in range(nsl):
                    i = i0 + di
                    kv_ps = psum_s.tile([P, 2 * HD], F32, tag="ps")
                    for ko in range(KO):
                        nc.tensor.matmul(kv_ps[:],
                                         xT_sb[:, ko, i * P:(i + 1) * P],
                                         wkvT_sb[:, ko, :],
                                         start=(ko == 0), stop=(ko == KO - 1))
                    kv_sb = workp.tile([P, 2 * HD], F32, tag=f"kvsb{di}",
                                       name=f"kvsb{di}", bufs=1)
                    nc.scalar.copy(kv_sb[:], kv_ps[:])
                    nc.vector.tensor_copy(v_sb[:, i, 0:HD], kv_ps[:, HD:2 * HD])
                    scrk = normp.tile([P, HD], F32, tag="scrk")
                    nc.scalar.square(scrk[:], kv_sb[:, 0:HD])
                    nc.vector.tensor_reduce(m_k[:, di:di + 1], scrk[:],
                                            axis=mybir.AxisListType.X,
                                            op=AOP.add)
                    kv_sbs.append(kv_sb)
                nc.vector.tensor_scalar(m_k[:, 0:nsl], m_k[:, 0:nsl],
                                        1.0 / HD, EPS,
                                        op0=AOP.mult, op1=AOP.add)
                yk = normp.tile([P, 8], F32, tag="yk")
                rsqrt_newton(m_k[:, 0:nsl], yk[:, 0:nsl], nsl)
                for di in range(nsl):
                    i = i0 + di
                    kn = workp.tile([P, 1, HD], F32, tag="kn")
                    nc.vector.tensor_scalar(kn[:, 0, :], kv_sbs[di][:, 0:HD],
                                            yk[:, di:di + 1], None,
                                            op0=AOP.mult)
                    rk = workp.tile([P, 1, HD], F32, tag="rk")
                    rope(rk, kn, "ck", "sk", i, 1)
                    tk_ps = psum_s.tile([HD, P], F32, tag="ps")
                    nc.tensor.transpose(tk_ps[:], rk[:, 0, :], ident[:])
                    nc.vector.tensor_copy(kT_sb[0:HD, i * P:(i + 1) * P],
                                          tk_ps[:])
                    nc.vector.tensor_copy(kT_sb[HD:P, i * P:(i + 1) * P],
                                          tk_ps[:])

            def q_batch(i0, nsl, qT_dst, d0):
                """Q proj + rmsnorm/rope + transpose for slices [i0,i0+nsl);
                batched Newton. Writes qT_dst at slice offset d0."""
                q_sbs = []
                m_q = normp.tile([P, 4 * HEADS_PER_CORE], F32, tag="mq")
                for di in range(nsl):
                    i = i0 + di
                    q_ps = psum_s.tile([P, DC], F32, tag="ps")
                    for ko in range(KO):
                        nc.tensor.matmul(q_ps[:],
                                         xT_sb[:, ko, i * P:(i + 1) * P],
                                         wqT_sb[:, ko, :],
                                         start=(ko == 0), stop=(ko == KO - 1))
                    q_sb = workp.tile([P, DC], F32, tag=f"qsb{di}",
                                      name=f"qsb{di}", bufs=1)
                    nc.vector.tensor_copy(q_sb[:], q_ps[:])
                    scr = normp.tile([P, HEADS_PER_CORE, HD], F32, tag="scr")
                    nc.vector.tensor_tensor(
                        scr[:], q_sb[:].rearrange("p (h d) -> p h d", d=HD),
                        q_sb[:].rearrange("p (h d) -> p h d", d=HD),
                        op=AOP.mult)
                    nc.vector.tensor_reduce(
                        m_q[:, di * HEADS_PER_CORE:(di + 1) * HEADS_PER_CORE],
                        scr[:], axis=mybir.AxisListType.X, op=AOP.add)
                    q_sbs.append(q_sb)
                w = nsl * HEADS_PER_CORE
                nc.vector.tensor_scalar(m_q[:, 0:w], m_q[:, 0:w], 1.0 / HD,
                                        EPS, op0=AOP.mult, op1=AOP.add)
                yq = normp.tile([P, 4 * HEADS_PER_CORE], F32, tag="yq")
                rsqrt_newton(m_q[:, 0:w], yq[:, 0:w], w)
                for di in range(nsl):
                    i = i0 + di
                    qn = workp.tile([P, HEADS_PER_CORE, HD], F32, tag="qn")
                    for h in range(HEADS_PER_CORE):
                        nc.vector.tensor_scalar(
                            qn[:, h, :], q_sbs[di][:, h * HD:(h + 1) * HD],
                            yq[:, di * HEADS_PER_CORE + h:
                               di * HEADS_PER_CORE + h + 1],
                            None, op0=AOP.mult)
                    rq = workp.tile([P, HEADS_PER_CORE, HD], F32, tag="rq")
                    rope(rq, qn, "cq", "sq", i, HEADS_PER_CORE)
                    for mc in range(2):
                        t_ps = psum_s.tile([P, P], F32, tag="ps")
                        nc.tensor.transpose(t_ps[:],
                                            rq[:, 2 * mc:2 * mc + 2, :],
                                            ident[:])
                        nc.vector.tensor_copy(
                            qT_dst[:, mc, (d0 + di) * P:(d0 + di + 1) * P],
                            t_ps[:])

            def attn(hp, tp):
                """Attention for head pair hp over t chunk tp.

                Returns outT tile [128, tw]: rows 0-63 head 2hp, 64-127
                head 2hp+1 (o-proj lhsT layout)."""
                t0 = tp * tc_per_tp * 512
                tw = tc_per_tp * 512
                qT_sb = qT_tp[tp]
                ow = outp.tile([P, tw], PV_DT, tag=f"ot_{hp}_{tp}",
                               name=f"ot_{hp}_{tp}")
                pv_ps = [psum_s.tile([P, 512], F32, tag="ps",
                                     name=f"pvps{_j}")
                         for _j in range(2 * tc_per_tp)]
                if mask_mode == "causal":
                    s_list = [s for s in range(ns) if s * P <= t0 + tw - 1]
                else:
                    s_list = list(range(ns))
                for si, s in enumerate(s_list):
                    qk_ps = psum_qk.tile([P, 2, tc_per_tp, 512], F32, tag="qk")
                    for j in range(2):
                        for tci in range(tc_per_tp):
                            nc.tensor.matmul(
                                qk_ps[:, j, tci, :],
                                kT_sb[HD * j:HD * (j + 1), s * P:(s + 1) * P],
                                qT_sb[HD * j:HD * (j + 1), hp,
                                      tci * 512:(tci + 1) * 512],
                                start=True, stop=True,
                                tile_position=(HD * j, 0))
                    tb = tbp.tile([P, 2, tc_per_tp, 512], F32, tag="tb")
                    nc.scalar.activation(tb[:], qk_ps[:], AF.Tanh,
                                         scale=1.0 / (8.0 * SOFTCAP))
                    if mask_mode != "none":
                        if mask_mode == "general" or s * P + P > t0:
                            mt = stagep.tile([P, tc_per_tp, 512], F32, tag="mt")
                            nc.sync.dma_start(
                                mt[:], maskT_d[s * P:(s + 1) * P, t0:t0 + tw]
                                .rearrange("p (c f) -> p c f", f=512))
                            nc.vector.tensor_tensor(
                                tb[:], tb[:],
                                mt[:, None, :, :].to_broadcast(
                                    (P, 2, tc_per_tp, 512)),
                                op=AOP.add)
                    pb = pbp.tile([P, 2, tc_per_tp, 512], PV_DT, tag="pb")
                    nc.scalar.activation(pb[:], tb[:], AF.Exp, scale=SOFTCAP)
                    for j in range(2):
                        for tci in range(tc_per_tp):
                            nc.tensor.matmul(
                                pv_ps[j * tc_per_tp + tci][0:HD + 1, :],
                                v_sb[:, s, 0:HD + 1],
                                pb[:, j, tci, :],
                                start=(si == 0), stop=(si == len(s_list) - 1))
                # normalize -> outT
                for j in range(2):
                    for tci in range(tc_per_tp):
                        jt = j * tc_per_tp + tci
                        rb = stagep.tile([HD, 512], F32, tag="rb", bufs=2)
                        nc.vector.reciprocal(rb[0:1, :],
                                             pv_ps[jt][HD:HD + 1, :])
                        nc.gpsimd.partition_broadcast(rb[:], rb[0:1, :],
                                                      channels=HD)
                        nc.vector.tensor_tensor(
                            ow[HD * j:HD * (j + 1),
                               tci * 512:(tci + 1) * 512],
                            pv_ps[jt][0:HD, :], rb[:], op=AOP.mult)
                return ow

            def oproj(ow_by_hp, tp):
                for ii in range(tc_per_tp * 4):
                    gi = tp * tc_per_tp * 4 + ii
                    for nh in range(2):
                        op_ps = psum_s.tile([P, 512], F32, tag="ps")
                        for ko in range(2):
                            nc.tensor.matmul(
                                op_ps[:],
                                ow_by_hp[ko][:, ii * P:(ii + 1) * P],
                                woT_sb[:, ko, nh * 512:(nh + 1) * 512],
                                start=(ko == 0), stop=(ko == 1))
                        o_sb = stagep.tile([P, 512], F32, tag="osb", bufs=3)
                        nc.vector.tensor_copy(o_sb[:], op_ps[:])
                        nc.sync.dma_start(
                            y_d.rearrange("(o p) e -> p o e",
                                          p=P)[:, gi, nh * 512:(nh + 1) * 512],
                            o_sb[:])

            # ---- emission: kv first, then per-tp attn with q(tp+1)
            # and oproj(tp) trailing (they fill scheduler gaps) ----
            for i0 in range(0, nt, 8):
                kv_batch(i0, min(8, nt - i0))
            for d0 in range(0, nt_per_tp, 4):
                q_batch(d0, min(4, nt_per_tp - d0), qT_tp[0], d0)
            prev = None
            for tp in range(ntp):
                ow_by_hp = [attn(hp, tp) for hp in range(2)]
                if tp + 1 < ntp:
                    for d0 in range(0, nt_per_tp, 4):
                        q_batch((tp + 1) * nt_per_tp + d0,
                                min(4, nt_per_tp - d0), qT_tp[tp + 1], d0)
                oproj(ow_by_hp, tp)
            del prev

    nc.finalize()
    return nc


def _get_nc(t_len, mask_mode):
    key = (t_len, mask_mode)
    if key not in _CACHE:
        _CACHE[key] = _build(t_len, mask_mode)
    return _CACHE[key]


def _host_prep(x, cos, sin, mask, wq, wk, wv, wo, q_norm_w, k_norm_w, t_len):
    f = np.float32
    wq, wk, wv, wo = (np.asarray(a, f) for a in (wq, wk, wv, wo))
    x = np.asarray(x, f)
    cos, sin = np.asarray(cos, f), np.asarray(sin, f)
    qw, kw = np.asarray(q_norm_w, f), np.asarray(k_norm_w, f)

    def eff(w):
        alpha = np.mean(np.abs(w), dtype=f)
        return (np.sign(w) * alpha).astype(f)

    wqe, wke, wve, woe = eff(wq), eff(wk), eff(wv), eff(wo)

    qw_sw = np.concatenate([qw[HH:], qw[:HH]])
    kw_sw = np.concatenate([kw[HH:], kw[:HH]])
    cosq = np.ascontiguousarray(cos * qw[None, :])
    sinq = np.ascontiguousarray(sin * qw_sw[None, :])
    cosk = np.ascontiguousarray(cos * kw[None, :])
    sink = np.ascontiguousarray(sin * kw_sw[None, :])

    m2 = np.asarray(mask, f).reshape(t_len, t_len)
    if not np.any(m2):
        mask_mode = "none"
        maskT = None
    else:
        causal = np.array_equal(
            m2, np.where(np.tril(np.ones((t_len, t_len), bool)), f(0), f(-1e9)))
        mask_mode = "causal" if causal else "general"
        maskT = np.ascontiguousarray(m2.T) / f(SOFTCAP)

    in_maps = []
    for c in range(N_CORES):
        b, g = divmod(c, KVH)
        im = {
            "xT": np.ascontiguousarray(x[b].T),
            "wqT": np.ascontiguousarray(wqe[g * DC:(g + 1) * DC, :].T),
            "wkvT": np.ascontiguousarray(
                np.concatenate([wke[g * HD:(g + 1) * HD, :],
                                wve[g * HD:(g + 1) * HD, :]], axis=0).T),
            "woT": np.ascontiguousarray(woe.T[g * DC:(g + 1) * DC, :]),
            "cosq": cosq, "sinq": sinq, "cosk": cosk, "sink": sink,
        }
        if maskT is not None:
            im["maskT"] = maskT
        in_maps.append(im)
    return in_maps, mask_mode


def kernel(x, cos, sin, mask, wq, wk, wv, wo, q_norm_w, k_norm_w,
           _trace=False, _t_len=T):
    in_maps, mask_mode = _host_prep(x, cos, sin, mask, wq, wk, wv, wo,
                                    q_norm_w, k_norm_w, _t_len)
    nc = _get_nc(_t_len, mask_mode)
    res = run_bass_kernel_spmd(nc, in_maps, core_ids=list(range(N_CORES)),
                               trace=_trace)
    out = np.zeros((B, _t_len, D), np.float32)
    for c in range(N_CORES):
        b = c // KVH
        out[b] += res.results[c]["y"]
    if _trace:
        kernel._last = res
    return out


# revision 21
# speedup vs baseline: 1.2664x; 1.0125x over previous
"""GQA attention with BitLinear projections, RMSNorm+RoPE, tanh softcap.

Sharding: 8 cores = batch(2) x kv-group(4). Each core handles one batch
element and one kv head (+ its 4 query heads), computes a partial o-proj
against its 256 columns of wo, and the host sums the 8 partials.
"""

import sys

if "/opt/trn_rl_repo" not in sys.path:
    sys.path.insert(0, "/opt/trn_rl_repo")

import numpy as np

import concourse.bass as bass
import concourse.mybir as mybir
import concourse.tile as tile
from concourse import bacc
from concourse.bass_utils import run_bass_kernel_spmd
from concourse.masks import make_identity

B, T, D, H, KVH, HD = 2, 2048, 1024, 16, 4, 64
HEADS_PER_CORE = H // KVH  # 4
DC = HEADS_PER_CORE * HD  # 256 q-proj dim per core
N_CORES = 8
SOFTCAP = 50.0
EPS = 1e-6
P = 128
HH = HD // 2

F32 = mybir.dt.float32
F32R = mybir.dt.float32r
I32 = mybir.dt.int32

QK_DT = F32R   # qT/kT tiles
PV_DT = F32R   # p and v tiles
PJ_DT = F32R   # x / projection weights
MAGIC = 0x5F375A86

_CACHE = {}


def _build(t_len, mask_mode):
    """mask_mode: 'none' | 'causal' | 'general'."""
    nt = t_len // P          # 128-row t slices
    ntc = t_len // 512       # 512-col t tiles
    ntp = max(t_len // 1024, 1)  # t chunk pairs (1024)
    tc_per_tp = ntc // ntp
    nt_per_tp = nt // ntp
    ns = t_len // P          # s chunks
    KO = D // P              # 8 contraction chunks
    AOP = mybir.AluOpType

    nc = bacc.Bacc(None, target_bir_lowering=False)

    xT_d = nc.dram_tensor("xT", [D, t_len], PJ_DT, kind="ExternalInput")
    wqT_d = nc.dram_tensor("wqT", [D, DC], PJ_DT, kind="ExternalInput")
    wkvT_d = nc.dram_tensor("wkvT", [D, 2 * HD], PJ_DT, kind="ExternalInput")
    woT_d = nc.dram_tensor("woT", [DC, D], PJ_DT, kind="ExternalInput")
    cosq_d = nc.dram_tensor("cosq", [t_len, HD], F32, kind="ExternalInput")
    sinq_d = nc.dram_tensor("sinq", [t_len, HD], F32, kind="ExternalInput")
    cosk_d = nc.dram_tensor("cosk", [t_len, HD], F32, kind="ExternalInput")
    sink_d = nc.dram_tensor("sink", [t_len, HD], F32, kind="ExternalInput")
    if mask_mode != "none":
        # mask transposed to [s, t] and divided by SOFTCAP on host
        maskT_d = nc.dram_tensor("maskT", [t_len, t_len], F32,
                                 kind="ExternalInput")
    y_d = nc.dram_tensor("y", [t_len, D], F32, kind="ExternalOutput")

    AF = mybir.ActivationFunctionType

    with tile.TileContext(nc) as tc:
        with (
            tc.tile_pool(name="const", bufs=1) as constp,
            tc.tile_pool(name="big", bufs=1) as bigp,
            tc.tile_pool(name="work", bufs=2) as workp,
            tc.tile_pool(name="normp", bufs=2) as normp,
            tc.tile_pool(name="tbp", bufs=2) as tbp,
            tc.tile_pool(name="pbp", bufs=2) as pbp,
            tc.tile_pool(name="outp", bufs=1) as outp,
            tc.tile_pool(name="stage", bufs=2) as stagep,
            tc.tile_pool(name="psum_s", bufs=4, space="PSUM") as psum_s,
            tc.tile_pool(name="psum_qk", bufs=1, space="PSUM") as psum_qk,
        ):
            ident = constp.tile([P, P], F32)
            make_identity(nc, ident)

            # ---- persistent loads ----
            xT_sb = bigp.tile([P, KO, t_len], PJ_DT, tag="xT")
            xT_r = xT_d.rearrange("(o p) t -> p o t", p=P)
            for ko in range(KO):
                nc.sync.dma_start(xT_sb[:, ko, :], xT_r[:, ko, :])
            wqT_sb = bigp.tile([P, KO, DC], PJ_DT, tag="wqT")
            nc.sync.dma_start(wqT_sb[:], wqT_d.rearrange("(o p) d -> p o d", p=P))
            wkvT_sb = bigp.tile([P, KO, 2 * HD], PJ_DT, tag="wkvT")
            nc.sync.dma_start(wkvT_sb[:], wkvT_d.rearrange("(o p) d -> p o d", p=P))
            woT_sb = bigp.tile([P, 2, D], PJ_DT, tag="woT")
            nc.sync.dma_start(woT_sb[:], woT_d.rearrange("(o p) e -> p o e", p=P))
            cs_sb = {}
            for name, dram in (("cq", cosq_d), ("sq", sinq_d),
                               ("ck", cosk_d), ("sk", sink_d)):
                cs_sb[name] = bigp.tile([P, nt, HD], F32, tag=name, name=name)
                nc.sync.dma_start(cs_sb[name][:],
                                  dram.rearrange("(o p) d -> p o d", p=P))

            qT_tp = [bigp.tile([P, 2, nt_per_tp * P], QK_DT, tag=f"qT{tp}",
                               name=f"qT{tp}") for tp in range(ntp)]
            kT_sb = bigp.tile([P, t_len], QK_DT, tag="kT")
            v_sb = bigp.tile([P, ns, HD + 2], PV_DT, tag="v")
            nc.vector.memset(v_sb[:].bitcast(F32), 1.0)

            magic = constp.tile([P, 16], I32, name="magic")
            nc.vector.memset(magic[:], MAGIC)

            def rsqrt_newton(m_ap, y_tile, width):
                """y = rsqrt(m) via bit-trick seed + 3 Newton iters (DVE)."""
                y_int = y_tile[:].bitcast(I32)
                nc.vector.tensor_scalar(y_int, m_ap.bitcast(I32), 1, None,
                                        op0=AOP.logical_shift_right)
                nc.vector.tensor_tensor(y_int, magic[:, 0:width], y_int,
                                        op=AOP.subtract)
                t1 = normp.tile([P, 16], F32, tag="t1")
                t1 = t1[:, 0:width]
                for _ in range(3):
                    nc.vector.tensor_tensor(t1, y_tile[:], y_tile[:],
                                            op=AOP.mult)
                    nc.vector.tensor_tensor(t1, m_ap, t1, op=AOP.mult)
                    nc.vector.tensor_scalar(t1, t1, -0.5, 1.5,
                                            op0=AOP.mult, op1=AOP.add)
                    nc.vector.tensor_tensor(y_tile[:], y_tile[:], t1,
                                            op=AOP.mult)

            def rope(dst, src, cn, sn, i, nh):
                """dst = src*cos + rotate_half(src)*sin; src [P, nh, HD]."""
                ta = workp.tile([P, HEADS_PER_CORE, HH], F32, tag="ta")
                ta = ta[:, 0:nh, :]
                bc = lambda ap: ap.to_broadcast((P, nh, HH)) if nh > 1 else ap
                c_lo = bc(cs_sb[cn][:, i:i + 1, 0:HH])
                s_lo = bc(cs_sb[sn][:, i:i + 1, 0:HH])
                c_hi = bc(cs_sb[cn][:, i:i + 1, HH:HD])
                s_hi = bc(cs_sb[sn][:, i:i + 1, HH:HD])
                nc.vector.tensor_tensor(dst[:, :, 0:HH], src[:, :, 0:HH], c_lo,
                                        op=AOP.mult)
                nc.vector.tensor_tensor(ta, src[:, :, HH:HD], s_lo, op=AOP.mult)
                nc.vector.tensor_tensor(dst[:, :, 0:HH], dst[:, :, 0:HH], ta,
                                        op=AOP.subtract)
                nc.vector.tensor_tensor(dst[:, :, HH:HD], src[:, :, HH:HD],
                                        c_hi, op=AOP.mult)
                nc.vector.tensor_tensor(ta, src[:, :, 0:HH], s_hi, op=AOP.mult)
                nc.vector.tensor_tensor(dst[:, :, HH:HD], dst[:, :, HH:HD], ta,
                                        op=AOP.add)

            def kv_batch(i0, nsl):
                """KV proj + k rmsnorm/rope + kT dup + v for slices
                [i0, i0+nsl); one batched Newton rsqrt for the whole batch.
                Copies/squares go on the otherwise-idle ScalarE."""
                kv_sbs = []
                m_k = normp.tile([P, 8], F32, tag="mk")
                for di in range(nsl):
                    i = i0 + di
                    kv_ps = psum_s.tile([P, 2 * HD], F32, tag="ps")
                    for ko in range(KO):
                        nc.tensor.matmul(kv_ps[:],
                                         xT_sb[:, ko, i * P:(i + 1) * P],
                                         wkvT_sb[:, ko, :],
                                         start=(ko == 0), stop=(ko == KO - 1))
                    kv_sb = workp.tile([P, 2 * HD], F32, tag=f"kvsb{di}",
                                       name=f"kvsb{di}", bufs=1)
                    nc.scalar.copy(kv_sb[:], kv_ps[:])
                    nc.vector.tensor_copy(v_sb[:, i, 0:HD], kv_ps[:, HD:2 * HD])
                    scrk = normp.tile([P, HD], F32, tag="scrk")
                    nc.scalar.square(scrk[:], kv_sb[:, 0:HD])
                    nc.vector.tensor_reduce(m_k[:, di:di + 1], scrk[:],
                                            axis=mybir.AxisListType.X,
                                            op=AOP.add)
                    kv_sbs.append(kv_sb)
                nc.vector.tensor_scalar(m_k[:, 0:nsl], m_k[:, 0:nsl],
                                        1.0 / HD, EPS,
                                        op0=AOP.mult, op1=AOP.add)
                yk = normp.tile([P, 8], F32, tag="yk")
                rsqrt_newton(m_k[:, 0:nsl], yk[:, 0:nsl], nsl)
                for di in range(nsl):
                    i = i0 + di
                    kn = workp.tile([P, 1, HD], F32, tag="kn")
                    nc.vector.tensor_scalar(kn[:, 0, :], kv_sbs[di][:, 0:HD],
                                            yk[:, di:di + 1], None,
                                            op0=AOP.mult)
                    rk = workp.tile([P, 1, HD], F32, tag="rk")
                    rope(rk, kn, "ck", "sk", i, 1)
                    tk_ps = psum_s.tile([HD, P], F32, tag="ps")
                    nc.tensor.transpose(tk_ps[:], rk[:, 0, :], ident[:])
                    nc.vector.tensor_copy(kT_sb[0:HD, i * P:(i + 1) * P],
                                          tk_ps[:])
                    nc.vector.tensor_copy(kT_sb[HD:P, i * P:(i + 1) * P],
                                          tk_ps[:])

            def q_batch(i0, nsl, qT_dst, d0):
                """Q proj + rmsnorm/rope + transpose for slices [i0,i0+nsl);
                batched Newton. Writes qT_dst at slice offset d0."""
                q_sbs = []
                m_q = normp.tile([P, 4 * HEADS_PER_CORE], F32, tag="mq")
                for di in range(nsl):
                    i = i0 + di
                    q_ps = psum_s.tile([P, DC], F32, tag="ps")
                    for ko in range(KO):
                        nc.tensor.matmul(q_ps[:],
                                         xT_sb[:, ko, i * P:(i + 1) * P],
                                         wqT_sb[:, ko, :],
                                         start=(ko == 0), stop=(ko == KO - 1))
                    q_sb = workp.tile([P, DC], F32, tag=f"qsb{di}",
                                      name=f"qsb{di}", bufs=1)
                    nc.scalar.copy(q_sb[:], q_ps[:])
                    scr = normp.tile([P, HEADS_PER_CORE, HD], F32, tag="scr")
                    nc.scalar.square(
                        scr[:].rearrange("p h d -> p (h d)"), q_sb[:])
                    nc.vector.tensor_reduce(
                        m_q[:, di * HEADS_PER_CORE:(di + 1) * HEADS_PER_CORE],
                        scr[:], axis=mybir.AxisListType.X, op=AOP.add)
                    q_sbs.append(q_sb)
                w = nsl * HEADS_PER_CORE
                nc.vector.tensor_scalar(m_q[:, 0:w], m_q[:, 0:w], 1.0 / HD,
                                        EPS, op0=AOP.mult, op1=AOP.add)
                yq = normp.tile([P, 4 * HEADS_PER_CORE], F32, tag="yq")
                rsqrt_newton(m_q[:, 0:w], yq[:, 0:w], w)
                for di in range(nsl):
                    i = i0 + di
                    qn = workp.tile([P, HEADS_PER_CORE, HD], F32, tag="qn")
                    for h in range(HEADS_PER_CORE):
                        nc.vector.tensor_scalar(
                            qn[:, h, :], q_sbs[di][:, h * HD:(h + 1) * HD],
                            yq[:, di * HEADS_PER_CORE + h:
                               di * HEADS_PER_CORE + h + 1],
                            None, op0=AOP.mult)
                    rq = workp.tile([P, HEADS_PER_CORE, HD], F32, tag="rq")
                    rope(rq, qn, "cq", "sq", i, HEADS_PER_CORE)
                    for mc in range(2):
                        t_ps = psum_s.tile([P, P], F32, tag="ps")
                        nc.tensor.transpose(t_ps[:],
                                            rq[:, 2 * mc:2 * mc + 2, :],
                                            ident[:])
                        nc.vector.tensor_copy(
                            qT_dst[:, mc, (d0 + di) * P:(d0 + di + 1) * P],
                            t_ps[:])

            def attn(hp, tp):
                """Attention for head pair hp over t chunk tp.

                Returns outT tile [128, tw]: rows 0-63 head 2hp, 64-127
                head 2hp+1 (o-proj lhsT layout)."""
                t0 = tp * tc_per_tp * 512
                tw = tc_per_tp * 512
                qT_sb = qT_tp[tp]
                ow = outp.tile([P, tw], PV_DT, tag=f"ot_{hp}_{tp}",
                               name=f"ot_{hp}_{tp}")
                pv_ps = [psum_s.tile([P, 512], F32, tag="ps",
                                     name=f"pvps{_j}")
                         for _j in range(2 * tc_per_tp)]
                if mask_mode == "causal":
                    s_list = [s for s in range(ns) if s * P <= t0 + tw - 1]
                else:
                    s_list = list(range(ns))
                for si, s in enumerate(s_list):
                    qk_ps = psum_qk.tile([P, 2, tc_per_tp, 512], F32, tag="qk")
                    for j in range(2):
                        for tci in range(tc_per_tp):
                            nc.tensor.matmul(
                                qk_ps[:, j, tci, :],
                                kT_sb[HD * j:HD * (j + 1), s * P:(s + 1) * P],
                                qT_sb[HD * j:HD * (j + 1), hp,
                                      tci * 512:(tci + 1) * 512],
                                start=True, stop=True,
                                tile_position=(HD * j, 0))
                    tb = tbp.tile([P, 2, tc_per_tp, 512], F32, tag="tb")
                    nc.scalar.activation(tb[:], qk_ps[:], AF.Tanh,
                                         scale=1.0 / (8.0 * SOFTCAP))
                    if mask_mode != "none":
                        if mask_mode == "general" or s * P + P > t0:
                            mt = stagep.tile([P, tc_per_tp, 512], F32, tag="mt")
                            nc.sync.dma_start(
                                mt[:], maskT_d[s * P:(s + 1) * P, t0:t0 + tw]
                                .rearrange("p (c f) -> p c f", f=512))
                            nc.vector.tensor_tensor(
                                tb[:], tb[:],
                                mt[:, None, :, :].to_broadcast(
                                    (P, 2, tc_per_tp, 512)),
                                op=AOP.add)
                    pb = pbp.tile([P, 2, tc_per_tp, 512], PV_DT, tag="pb")
                    nc.scalar.activation(pb[:], tb[:], AF.Exp, scale=SOFTCAP)
                    for j in range(2):
                        for tci in range(tc_per_tp):
                            nc.tensor.matmul(
                                pv_ps[j * tc_per_tp + tci][0:HD + 1, :],
                                v_sb[:, s, 0:HD + 1],
                                pb[:, j, tci, :],
                                start=(si == 0), stop=(si == len(s_list) - 1))
                # normalize -> outT
                for j in range(2):
                    for tci in range(tc_per_tp):
                        jt = j * tc_per_tp + tci
                        rb = stagep.tile([HD, 512], F32, tag="rb", bufs=2)
                        nc.vector.reciprocal(rb[0:1, :],
                                             pv_ps[jt][HD:HD + 1, :])
                        nc.gpsimd.partition_broadcast(rb[:], rb[0:1, :],
                                                      channels=HD)
                        nc.vector.tensor_tensor(
                            ow[HD * j:HD * (j + 1),
                               tci * 512:(tci + 1) * 512],
                            pv_ps[jt][0:HD, :], rb[:], op=AOP.mult)
                return ow

            def oproj(ow_by_hp, tp):
                for ii in range(tc_per_tp * 4):
                    gi = tp * tc_per_tp * 4 + ii
                    for nh in range(2):
                        op_ps = psum_s.tile([P, 512], F32, tag="ps")
                        for ko in range(2):
                            nc.tensor.matmul(
                                op_ps[:],
                                ow_by_hp[ko][:, ii * P:(ii + 1) * P],
                                woT_sb[:, ko, nh * 512:(nh + 1) * 512],
                                start=(ko == 0), stop=(ko == 1))
                        o_sb = stagep.tile([P, 512], F32, tag="osb", bufs=3)
                        nc.vector.tensor_copy(o_sb[:], op_ps[:])
                        nc.sync.dma_start(
                            y_d.rearrange("(o p) e -> p o e",
                                          p=P)[:, gi, nh * 512:(nh + 1) * 512],
                            o_sb[:])

            # ---- emission: kv first, then per-tp attn with q(tp+1)
            # and oproj(tp) trailing (they fill scheduler gaps) ----
            for i0 in range(0, nt, 8):
                kv_batch(i0, min(8, nt - i0))
            for i0 in range(0, nt, 4):
                q_batch(i0, min(4, nt - i0), qT_tp[i0 // nt_per_tp],
                        i0 % nt_per_tp)
            for tp in range(ntp):
                ow_by_hp = [attn(hp, tp) for hp in range(2)]
                oproj(ow_by_hp, tp)

    nc.finalize()
    return nc


def _get_nc(t_len, mask_mode):
    key = (t_len, mask_mode)
    if key not in _CACHE:
        _CACHE[key] = _build(t_len, mask_mode)
    return _CACHE[key]


def _host_prep(x, cos, sin, mask, wq, wk, wv, wo, q_norm_w, k_norm_w, t_len):
    f = np.float32
    wq, wk, wv, wo = (np.asarray(a, f) for a in (wq, wk, wv, wo))
    x = np.asarray(x, f)
    cos, sin = np.asarray(cos, f), np.asarray(sin, f)
    qw, kw = np.asarray(q_norm_w, f), np.asarray(k_norm_w, f)

    def eff(w):
        alpha = np.mean(np.abs(w), dtype=f)
        return (np.sign(w) * alpha).astype(f)

    wqe, wke, wve, woe = eff(wq), eff(wk), eff(wv), eff(wo)

    qw_sw = np.concatenate([qw[HH:], qw[:HH]])
    kw_sw = np.concatenate([kw[HH:], kw[:HH]])
    cosq = np.ascontiguousarray(cos * qw[None, :])
    sinq = np.ascontiguousarray(sin * qw_sw[None, :])
    cosk = np.ascontiguousarray(cos * kw[None, :])
    sink = np.ascontiguousarray(sin * kw_sw[None, :])

    m2 = np.asarray(mask, f).reshape(t_len, t_len)
    if not np.any(m2):
        mask_mode = "none"
        maskT = None
    else:
        causal = np.array_equal(
            m2, np.where(np.tril(np.ones((t_len, t_len), bool)), f(0), f(-1e9)))
        mask_mode = "causal" if causal else "general"
        maskT = np.ascontiguousarray(m2.T) / f(SOFTCAP)

    in_maps = []
    for c in range(N_CORES):
        b, g = divmod(c, KVH)
        im = {
            "xT": np.ascontiguousarray(x[b].T),
            "wqT": np.ascontiguousarray(wqe[g * DC:(g + 1) * DC, :].T),
            "wkvT": np.ascontiguousarray(
                np.concatenate([wke[g * HD:(g + 1) * HD, :],
                                wve[g * HD:(g + 1) * HD, :]], axis=0).T),
            "woT": np.ascontiguousarray(woe.T[g * DC:(g + 1) * DC, :]),
            "cosq": cosq, "sinq": sinq, "cosk": cosk, "sink": sink,
        }
        if maskT is not None:
            im["maskT"] = maskT
        in_maps.append(im)
    return in_maps, mask_mode


def kernel(x, cos, sin, mask, wq, wk, wv, wo, q_norm_w, k_norm_w,
           _trace=False, _t_len=T):
    in_maps, mask_mode = _host_prep(x, cos, sin, mask, wq, wk, wv, wo,
                                    q_norm_w, k_norm_w, _t_len)
    nc = _get_nc(_t_len, mask_mode)
    res = run_bass_kernel_spmd(nc, in_maps, core_ids=list(range(N_CORES)),
                               trace=_trace)
    out = np.zeros((B, _t_len, D), np.float32)
    for c in range(N_CORES):
        b = c // KVH
        out[b] += res.results[c]["y"]
    if _trace:
        kernel._last = res
    return out
